# revision 15
# baseline (speedup 1.0000x reference)
"""Trainium2 Bass kernel for nn_NetworkActivity_layer (masked linear):

    out = x @ (weight * mask.T).T + bias      x:(4096,15000) w:(500,15000)
                                              mask:(15000,500) bias:(500,)

Strategy: shard the contraction (gene) dim K=15000 across 8 NeuronCores
(1875 genes/core). Each core computes a partial (4096,500) fp32 output:
    partial_i = x[:, sh_i] @ (weight[:, sh_i] * mask[sh_i, :].T).T
Host sums the 8 partials (the K-shard "unshard" step). The bias is folded
into an extra padded gene row (x column of ones, weight row = bias on core
0, mask row = 1), so the device kernel computes the complete affine map.

Per-core layout (host-packed for DMA friendliness + TensorE layout):
  genes padded 1875 -> 1920 = 15 k-tiles x 128 (FWL needs K=128 exactly)
  xt: (32, 128, 1920) bf16   xt[m, p, k*128+c] = xpad[m*128+c, k*128+p]
      -> SBUF tile [128, 1920]; slice [:, k*128:(k+1)*128] is the
         stationary lhsT (K=128 genes, M=128 batch) for (m, k)
  wt/mk: (128, 7500) bf16    [p, k*500+n] = wpad/mpad[k*128+p, n]
      -> masked weights mw = wt*mk computed on-device; slice
         [:, k*500:(k+1)*500] is the moving rhs (K=128, N=500)
  out: (32, 128, 500) fp32 partial, accumulated over 15 k-tiles in PSUM.
"""

import functools
import os

import ml_dtypes
import numpy as np

B, G, P = 4096, 15000, 500
LAMBDA = 0.1  # mask value for non-annotated gene/pathway pairs
N_CORES = 8
GS = G // N_CORES          # 1875 genes per core
KT = 128                   # k-tile size (partition dim; 128 enables FWL)
NK = 15                    # k-tiles per core
KP = NK * KT               # 1920 padded genes (row GS=1875 carries bias)
MT = 128                   # batch tile
NM = B // MT               # 32 batch tiles

_BF16 = ml_dtypes.bfloat16

LAST_EXEC_TIME_NS = None
LAST_TRACE = None
LAST_RESULTS = None


def _install_profshim():
    """Make run_bass_kernel_spmd(trace=True) work in the axon container:
    recreate the antenv.axon_hooks NTFF hook + keep artifacts local."""
    import sys
    import types

    if "antenv.axon_hooks" not in sys.modules:
        import antenv
        from trn_agent_boot.trn_boot import _ntff_profile_via_ctypes

        mod = types.ModuleType("antenv.axon_hooks")
        mod._hook = _ntff_profile_via_ctypes("/opt/axon/libaxon_pjrt.so")
        mod.set_axon_ntff_profile_hook = lambda h: setattr(mod, "_hook", h)
        mod.get_axon_ntff_profile_hook = lambda: mod._hook
        sys.modules["antenv.axon_hooks"] = mod
        antenv.axon_hooks = mod

    import concourse.bass_utils as bu

    bu.upload_artifacts = lambda tmpdir: f"file://{tmpdir}"


@functools.lru_cache(maxsize=1)
def _build():
    import concourse.bass as bass
    import concourse.mybir as mybir
    import concourse.tile as tile
    from concourse import bacc

    nc = bacc.Bacc(
        "TRN2", target_bir_lowering=False, debug=False, num_devices=N_CORES
    )
    bf16 = mybir.dt.bfloat16
    f32 = mybir.dt.float32
    NC_W = 3  # w/mask load chunks
    CH = NK // NC_W  # k-tiles per chunk
    xt_d = nc.dram_tensor("xt", [NM, KT, KP], bf16, kind="ExternalInput")
    wt_d = nc.dram_tensor("wt", [NC_W, KT, CH * P], bf16, kind="ExternalInput")
    # mask is exactly {lambda, 1.0}; ship it as uint8 {0,1} (half the
    # critical-path bytes) and reconstruct lambda + (1-lambda)*a on DVE
    mk_d = nc.dram_tensor("mk", [NC_W, KT, CH * P], mybir.dt.uint8, kind="ExternalInput")
    out_d = nc.dram_tensor("out", [NM, MT, P], f32, kind="ExternalOutput")

    with tile.TileContext(nc) as tc:
        with (
            tc.tile_pool(name="wpool", bufs=1) as wpool,
            tc.tile_pool(name="wstage", bufs=4) as wstage,
            tc.tile_pool(name="xpool", bufs=4) as xpool,
            tc.tile_pool(name="opool", bufs=3) as opool,
            tc.tile_pool(name="pspool", bufs=4, space=bass.MemorySpace.PSUM) as pspool,
        ):
            # Pre-warm the PE HAM clock gate during the initial weight-load
            # window: ~4us of junk matmuls on garbage data makes the 4096-cycle
            # activity window fire before the real matmuls start, so they run
            # at 2.4GHz instead of ramping from 1.2GHz.
            junk = wpool.tile([KT, 512], bf16)
            nc.gpsimd.memset(junk[:], 0.0)
            jps = pspool.tile([MT, 512], f32, tag="jps")
            for _ in range(10):
                nc.tensor.matmul(jps[:], junk[:, 0:128], junk[:], start=True, stop=True)

            mw = wpool.tile([KT, NK * P], bf16)
            # w/mask load on the Activation HWDGE ring (parallel to Sync's
            # xt stream), in 3 chunks of 5 k-tiles so the first matmuls
            # start after ~1/3 of the 3.8MB load; per-k muls on DVE give
            # matmul k its rhs as soon as its chunk lands.
            for c in range(NC_W):
                wt_c = wstage.tile([KT, CH * P], bf16, tag="wt_c")
                mk_c = wstage.tile([KT, CH * P], mybir.dt.uint8, tag="mk_c")
                nc.scalar.dma_start(mk_c[:], mk_d[c])
                nc.scalar.dma_start(wt_c[:], wt_d[c])
                for j in range(CH):
                    k = c * CH + j
                    mdec = wstage.tile([KT, P], bf16, tag="mdec")
                    nc.vector.tensor_scalar(
                        mdec[:],
                        mk_c[:, j * P : (j + 1) * P],
                        1.0 - LAMBDA,
                        LAMBDA,
                        mybir.AluOpType.mult,
                        mybir.AluOpType.add,
                    )
                    nc.vector.tensor_mul(
                        mw[:, k * P : (k + 1) * P],
                        wt_c[:, j * P : (j + 1) * P],
                        mdec[:],
                    )
            for m in range(NM):
                xt = xpool.tile([KT, KP], bf16)
                nc.sync.dma_start(xt[:], xt_d[m])
                ps = pspool.tile([MT, P], f32)
                for k in range(NK):
                    nc.tensor.matmul(
                        ps[:],
                        xt[:, k * MT : (k + 1) * MT],
                        mw[:, k * P : (k + 1) * P],
                        start=(k == 0),
                        stop=(k == NK - 1),
                    )
                ot = opool.tile([MT, P], f32)
                nc.vector.tensor_copy(ot[:], ps[:])
                nc.sync.dma_start(out_d[m], ot[:])
    nc.compile()
    return nc


def _pack_inputs(x, weight, mask, bias):
    """Host-side shard + pre-tile. Returns in_maps for the 8 cores."""
    xb = np.asarray(x, dtype=np.float32).astype(_BF16)  # (B, G) one cast pass
    wf = np.asarray(weight, dtype=np.float32)
    mf = np.asarray(mask, dtype=np.float32)
    bf = np.asarray(bias, dtype=np.float32)

    in_maps = []
    for core in range(N_CORES):
        g0 = core * GS
        xpad = np.zeros((B, KP), dtype=_BF16)
        xpad[:, :GS] = xb[:, g0 : g0 + GS]
        xpad[:, GS] = _BF16(1.0)  # bias column
        # [m, c, k, p] -> [m, p, k, c]
        xt = np.ascontiguousarray(
            xpad.reshape(NM, MT, NK, KT).transpose(0, 3, 2, 1)
        ).reshape(NM, KT, NK * MT)

        # chunk-major pack: wt[c, p, j*P+n] = wpad[(c*CH+j)*KT + p, n]
        NC_W, CH = 3, NK // 3
        wpad = np.zeros((KP, P), dtype=np.float32)
        wpad[:GS] = wf[:, g0 : g0 + GS].T
        if core == 0:
            wpad[GS] = bf  # bias row (counted exactly once across cores)
        wt = np.ascontiguousarray(
            wpad.reshape(NC_W, CH, KT, P).transpose(0, 2, 1, 3)
        ).reshape(NC_W, KT, CH * P).astype(_BF16)

        mpad = np.zeros((KP, P), dtype=np.float32)
        mpad[:GS] = mf[g0 : g0 + GS]
        mpad[GS] = 1.0
        mk = np.ascontiguousarray(
            (mpad >= 0.5).reshape(NC_W, CH, KT, P).transpose(0, 2, 1, 3)
        ).reshape(NC_W, KT, CH * P).astype(np.uint8)
        in_maps.append({"xt": xt, "wt": wt, "mk": mk})
    return in_maps


def kernel(x, weight, mask, bias):
    global LAST_EXEC_TIME_NS, LAST_TRACE, LAST_RESULTS

    profile = bool(int(os.environ.get("KERNEL_PROFILE", "0")))
    if profile:
        _install_profshim()

    nc = _build()
    in_maps = _pack_inputs(x, weight, mask, bias)

    from concourse.bass_utils import run_bass_kernel_spmd

    tmpdir = None
    if profile:
        import tempfile

        base = os.environ.get("KERNEL_TRACE_DIR")
        if base:
            os.makedirs(base, exist_ok=True)
        tmpdir = tempfile.mkdtemp(prefix="ktrace_", dir=base)

    res = run_bass_kernel_spmd(
        nc,
        in_maps,
        core_ids=list(range(N_CORES)),
        trace=profile,
        tmpdir=tmpdir,
    )
    LAST_EXEC_TIME_NS = res.exec_time_ns
    LAST_TRACE = (
        res.instructions_and_trace[1] if res.instructions_and_trace else None
    )
    LAST_RESULTS = res

    parts = np.stack([r["out"].reshape(B, P) for r in res.results])
    return parts.sum(axis=0, dtype=np.float32)


# revision 16
# speedup vs baseline: 1.0188x; 1.0188x over previous
"""Trainium2 Bass kernel for nn_NetworkActivity_layer (masked linear):

    out = x @ (weight * mask.T).T + bias      x:(4096,15000) w:(500,15000)
                                              mask:(15000,500) bias:(500,)

Strategy: shard the contraction (gene) dim K=15000 across 8 NeuronCores
(1875 genes/core). Each core computes a partial (4096,500) fp32 output:
    partial_i = x[:, sh_i] @ (weight[:, sh_i] * mask[sh_i, :].T).T
Host sums the 8 partials (the K-shard "unshard" step). The bias is folded
into an extra padded gene row (x column of ones, weight row = bias on core
0, mask row = 1), so the device kernel computes the complete affine map.

Per-core layout (host-packed for DMA friendliness + TensorE layout):
  genes padded 1875 -> 1920 = 15 k-tiles x 128 (FWL needs K=128 exactly)
  xt: (32, 128, 1920) bf16   xt[m, p, k*128+c] = xpad[m*128+c, k*128+p]
      -> SBUF tile [128, 1920]; slice [:, k*128:(k+1)*128] is the
         stationary lhsT (K=128 genes, M=128 batch) for (m, k)
  wt/mk: (128, 7500) bf16    [p, k*500+n] = wpad/mpad[k*128+p, n]
      -> masked weights mw = wt*mk computed on-device; slice
         [:, k*500:(k+1)*500] is the moving rhs (K=128, N=500)
  out: (32, 128, 500) fp32 partial, accumulated over 15 k-tiles in PSUM.
"""

import functools
import os

import ml_dtypes
import numpy as np

B, G, P = 4096, 15000, 500
LAMBDA = 0.1  # mask value for non-annotated gene/pathway pairs
N_CORES = 8
GS = G // N_CORES          # 1875 genes per core
KT = 128                   # k-tile size (partition dim; 128 enables FWL)
NK = 15                    # k-tiles per core
KP = NK * KT               # 1920 padded genes (row GS=1875 carries bias)
MT = 128                   # batch tile
NM = B // MT               # 32 batch tiles

_BF16 = ml_dtypes.bfloat16

LAST_EXEC_TIME_NS = None
LAST_TRACE = None
LAST_RESULTS = None


def _install_profshim():
    """Make run_bass_kernel_spmd(trace=True) work in the axon container:
    recreate the antenv.axon_hooks NTFF hook + keep artifacts local."""
    import sys
    import types

    if "antenv.axon_hooks" not in sys.modules:
        import antenv
        from trn_agent_boot.trn_boot import _ntff_profile_via_ctypes

        mod = types.ModuleType("antenv.axon_hooks")
        mod._hook = _ntff_profile_via_ctypes("/opt/axon/libaxon_pjrt.so")
        mod.set_axon_ntff_profile_hook = lambda h: setattr(mod, "_hook", h)
        mod.get_axon_ntff_profile_hook = lambda: mod._hook
        sys.modules["antenv.axon_hooks"] = mod
        antenv.axon_hooks = mod

    import concourse.bass_utils as bu

    bu.upload_artifacts = lambda tmpdir: f"file://{tmpdir}"


@functools.lru_cache(maxsize=1)
def _build():
    import concourse.bass as bass
    import concourse.mybir as mybir
    import concourse.tile as tile
    from concourse import bacc

    nc = bacc.Bacc(
        "TRN2", target_bir_lowering=False, debug=False, num_devices=N_CORES
    )
    bf16 = mybir.dt.bfloat16
    f32 = mybir.dt.float32
    NC_W = 3  # w/mask load chunks
    CH = NK // NC_W  # k-tiles per chunk
    xt_d = nc.dram_tensor("xt", [NM, KT, KP], bf16, kind="ExternalInput")
    wt_d = nc.dram_tensor("wt", [NC_W, KT, CH * P], bf16, kind="ExternalInput")
    # mask is exactly {lambda, 1.0}; ship it as uint8 {0,1} (half the
    # critical-path bytes) and reconstruct lambda + (1-lambda)*a on DVE
    mk_d = nc.dram_tensor("mk", [NC_W, KT, CH * P], mybir.dt.uint8, kind="ExternalInput")
    out_d = nc.dram_tensor("out", [NM, MT, P], f32, kind="ExternalOutput")

    with tile.TileContext(nc) as tc:
        with (
            tc.tile_pool(name="wpool", bufs=1) as wpool,
            tc.tile_pool(name="wstage", bufs=4) as wstage,
            tc.tile_pool(name="xpool", bufs=4) as xpool,
            tc.tile_pool(name="opool", bufs=3) as opool,
            tc.tile_pool(name="pspool", bufs=4, space=bass.MemorySpace.PSUM) as pspool,
        ):
            # Pre-warm the PE HAM clock gate during the initial weight-load
            # window: ~4us of junk matmuls on garbage data makes the 4096-cycle
            # activity window fire before the real matmuls start, so they run
            # at 2.4GHz instead of ramping from 1.2GHz.
            junk = wpool.tile([KT, 512], bf16)
            nc.gpsimd.memset(junk[:], 0.0)
            jps = pspool.tile([MT, 512], f32, tag="jps")
            for _ in range(17):
                nc.tensor.matmul(jps[:], junk[:, 0:128], junk[:], start=True, stop=True)

            mw = wpool.tile([KT, NK * P], bf16)
            # w/mask load on the Activation HWDGE ring (parallel to Sync's
            # xt stream), in 3 chunks of 5 k-tiles so the first matmuls
            # start after ~1/3 of the 3.8MB load; per-k muls on DVE give
            # matmul k its rhs as soon as its chunk lands.
            for c in range(NC_W):
                wt_c = wstage.tile([KT, CH * P], bf16, tag="wt_c")
                mk_c = wstage.tile([KT, CH * P], mybir.dt.uint8, tag="mk_c")
                nc.scalar.dma_start(mk_c[:], mk_d[c])
                nc.scalar.dma_start(wt_c[:], wt_d[c])
                for j in range(CH):
                    k = c * CH + j
                    mdec = wstage.tile([KT, P], bf16, tag="mdec")
                    nc.vector.tensor_scalar(
                        mdec[:],
                        mk_c[:, j * P : (j + 1) * P],
                        1.0 - LAMBDA,
                        LAMBDA,
                        mybir.AluOpType.mult,
                        mybir.AluOpType.add,
                    )
                    nc.vector.tensor_mul(
                        mw[:, k * P : (k + 1) * P],
                        wt_c[:, j * P : (j + 1) * P],
                        mdec[:],
                    )
            for m in range(NM):
                xt = xpool.tile([KT, KP], bf16)
                nc.sync.dma_start(xt[:], xt_d[m])
                ps = pspool.tile([MT, P], f32)
                for k in range(NK):
                    nc.tensor.matmul(
                        ps[:],
                        xt[:, k * MT : (k + 1) * MT],
                        mw[:, k * P : (k + 1) * P],
                        start=(k == 0),
                        stop=(k == NK - 1),
                    )
                ot = opool.tile([MT, P], f32)
                nc.vector.tensor_copy(ot[:], ps[:])
                nc.sync.dma_start(out_d[m], ot[:])
    nc.compile()
    return nc


def _pack_inputs(x, weight, mask, bias):
    """Host-side shard + pre-tile. Returns in_maps for the 8 cores."""
    xb = np.asarray(x, dtype=np.float32).astype(_BF16)  # (B, G) one cast pass
    wf = np.asarray(weight, dtype=np.float32)
    mf = np.asarray(mask, dtype=np.float32)
    bf = np.asarray(bias, dtype=np.float32)

    in_maps = []
    for core in range(N_CORES):
        g0 = core * GS
        xpad = np.zeros((B, KP), dtype=_BF16)
        xpad[:, :GS] = xb[:, g0 : g0 + GS]
        xpad[:, GS] = _BF16(1.0)  # bias column
        # [m, c, k, p] -> [m, p, k, c]
        xt = np.ascontiguousarray(
            xpad.reshape(NM, MT, NK, KT).transpose(0, 3, 2, 1)
        ).reshape(NM, KT, NK * MT)

        # chunk-major pack: wt[c, p, j*P+n] = wpad[(c*CH+j)*KT + p, n]
        NC_W, CH = 3, NK // 3
        wpad = np.zeros((KP, P), dtype=np.float32)
        wpad[:GS] = wf[:, g0 : g0 + GS].T
        if core == 0:
            wpad[GS] = bf  # bias row (counted exactly once across cores)
        wt = np.ascontiguousarray(
            wpad.reshape(NC_W, CH, KT, P).transpose(0, 2, 1, 3)
        ).reshape(NC_W, KT, CH * P).astype(_BF16)

        mpad = np.zeros((KP, P), dtype=np.float32)
        mpad[:GS] = mf[g0 : g0 + GS]
        mpad[GS] = 1.0
        mk = np.ascontiguousarray(
            (mpad >= 0.5).reshape(NC_W, CH, KT, P).transpose(0, 2, 1, 3)
        ).reshape(NC_W, KT, CH * P).astype(np.uint8)
        in_maps.append({"xt": xt, "wt": wt, "mk": mk})
    return in_maps


def kernel(x, weight, mask, bias):
    global LAST_EXEC_TIME_NS, LAST_TRACE, LAST_RESULTS

    profile = bool(int(os.environ.get("KERNEL_PROFILE", "0")))
    if profile:
        _install_profshim()

    nc = _build()
    in_maps = _pack_inputs(x, weight, mask, bias)

    from concourse.bass_utils import run_bass_kernel_spmd

    tmpdir = None
    if profile:
        import tempfile

        base = os.environ.get("KERNEL_TRACE_DIR")
        if base:
            os.makedirs(base, exist_ok=True)
        tmpdir = tempfile.mkdtemp(prefix="ktrace_", dir=base)

    res = run_bass_kernel_spmd(
        nc,
        in_maps,
        core_ids=list(range(N_CORES)),
        trace=profile,
        tmpdir=tmpdir,
    )
    LAST_EXEC_TIME_NS = res.exec_time_ns
    LAST_TRACE = (
        res.instructions_and_trace[1] if res.instructions_and_trace else None
    )
    LAST_RESULTS = res

    parts = np.stack([r["out"].reshape(B, P) for r in res.results])
    return parts.sum(axis=0, dtype=np.float32)
